# revision 24
# baseline (speedup 1.0000x reference)
"""GCN message-passing kernel for 8 TRN2 NeuronCores.

Reference computation (per (b, c) pair, fp32 reference):
    e1  = x @ W1^T + b1          [N, H]
    e2  = x @ W2^T + b2          [N, H]
    adj = relu(e1 @ e2^T)        [N, N]
    h   = adj @ x                [N, F]
    out = h @ W3^T + b3          [N, O]

Sharding: the 32 (b, c) pairs are split 4-per-core across 8 cores;
weights are replicated. Each core runs an identical Bass program fully
fused in SBUF/PSUM (the N x N adjacency never touches HBM).

All matmul operands are bf16 (PSUM accumulation stays fp32); the 2e-2
rel-err gate leaves ample room (bf16 quantization contributes ~3e-3).

Parity layout: every K=64 contraction is packed two-per-PE via
tile_position row tiles (0,*) / (64,*), which the PE executes
concurrently (measured 2x). Row-block t of a pair lives on partition
half t%2: the [128,128] x-transposes emit (f@t-even ; f@t-odd) stacked,
and e1/e2/h^T inherit the layout, so adjacency, the e-matmuls and the
output projection all pair up. biases ride along on the mandatory
PSUM->SBUF copies (per-partition bias on ACT, tensor-tensor add on DVE)
instead of K=65 augmentation, keeping contractions pairable at K=64.

Layout trick: partition p of SBUF holds rows [16p, 16p+16) of the pair
(a pure row permutation, applied consistently to both sides of every
contraction and undone by the output DMA), which makes every HBM
transfer contiguous 4KB per partition.
"""

import sys

for _p in ("/opt/trn_rl_repo",):
    if _p not in sys.path:
        sys.path.insert(0, _p)

import numpy as np

import concourse.bass as bass
import concourse.tile as tile
from concourse import bacc, mybir
from concourse.bass import ts
from concourse.masks import make_identity

B, C, N, F = 4, 8, 2048, 64
H = 64
O = 64
NCORES = 8
PAIRS = (B * C) // NCORES  # 4 (b,c) pairs per core
P = 128                    # SBUF partitions
HP = 64                    # half-partition (PE row/col tile)
TBLK = N // P              # 16 row-blocks per pair
NPAIR_T = TBLK // 2        # 8 block-pairs (even t, odd t)
CH = 512                   # moving-operand chunk (one PSUM bank of fp32)
NH = N // 2                # 1024 columns per parity
NCH2 = NH // CH            # 2 chunks per parity
F32 = mybir.dt.float32
BF16 = mybir.dt.bfloat16

AF = mybir.ActivationFunctionType
ID = AF.Identity


class _Rotor:
    """Alternate PSUM->SBUF relu/copy work across DVE and ACT.
    (GPSIMD cannot access PSUM on TRN2, so Pool only gets SBUF-only work.)"""

    def __init__(self, nc):
        self.nc = nc
        self.i = 0

    def relu(self, out, in_):
        e = "da"[self.i % 2]
        self.i += 1
        if e == "d":
            self.nc.vector.tensor_scalar_max(out, in_, 0.0)
        else:
            self.nc.scalar.activation(out, in_, AF.Relu)

    def copy(self, out, in_):
        e = "da"[self.i % 2]
        self.i += 1
        if e == "d":
            self.nc.vector.tensor_copy(out, in_)
        else:
            self.nc.scalar.copy(out, in_)


def _emit(tc, x_d, w_d, b_d, out_d, reps=1, variant="full"):
    nc = tc.nc
    rot = _Rotor(nc)

    import contextlib

    with contextlib.ExitStack() as ctx:
        consts = ctx.enter_context(tc.tile_pool(name="consts", bufs=1))
        xpool = ctx.enter_context(tc.tile_pool(name="xp", bufs=2))
        xtpool = ctx.enter_context(tc.tile_pool(name="xt", bufs=2))
        epool = ctx.enter_context(tc.tile_pool(name="ep", bufs=2))
        adjpool = ctx.enter_context(tc.tile_pool(name="adj", bufs=12))
        htpool = ctx.enter_context(tc.tile_pool(name="ht", bufs=2))
        opool = ctx.enter_context(tc.tile_pool(name="op", bufs=2))
        ps_adj = ctx.enter_context(tc.tile_pool(name="psa", bufs=6, space="PSUM"))
        ps_h = ctx.enter_context(tc.tile_pool(name="psh", bufs=2, space="PSUM"))

        ident = consts.tile([P, P], F32)
        make_identity(nc, ident[:])
        identb = consts.tile([P, P], BF16)
        make_identity(nc, identb[:])

        # Weights W1/W2/W3 transposed to [f, h] bf16, duplicated on both
        # partition halves (for row-tile pairing); per-partition bias cols.
        wstats, bcols = [], []
        for k in range(3):
            wraw = consts.tile([H, F], F32, tag="wraw")
            nc.sync.dma_start(wraw[:], w_d[k][:])
            wst = consts.tile([P, H], BF16, tag=f"wst{k}")
            pw = ps_adj.tile([F, H], F32, tag="pa")
            nc.tensor.transpose(pw[:], wraw[:], ident[0:H, 0:H])
            nc.vector.tensor_copy(wst[0:HP, :], pw[:])
            nc.sync.dma_start(wst[HP:P, :], wst[0:HP, :])
            wstats.append(wst)
            if k < 2:
                bcol = consts.tile([P, 1], F32, tag=f"bc{k}")
                nc.sync.dma_start(bcol[0:HP, :], b_d[k].unsqueeze(1))
                nc.sync.dma_start(bcol[HP:P, :], b_d[k].unsqueeze(1))
                bcols.append(bcol)
        w1st, w2st, w3st = wstats
        b1c, b2c = bcols

        # b3 replicated [128, 512] (cols = 8 blocks x 64 o) for the fused
        # bias-add on the projection's PSUM->SBUF copy.
        b3row = consts.tile([1, O], F32)
        nc.sync.dma_start(b3row[:], b_d[2].unsqueeze(0))
        b3x8 = consts.tile([1, CH], F32)
        for j in range(CH // O):
            nc.vector.tensor_copy(b3x8[:, ts(j, O)], b3row[:])
        ones1 = consts.tile([1, P], F32)
        nc.gpsimd.memset(ones1[:], 1.0)
        b3rep = consts.tile([P, CH], F32)
        pb3 = ps_adj.tile([P, CH], F32, tag="pa")
        nc.tensor.matmul(pb3[:], ones1[:], b3x8[:], start=True, stop=True)
        nc.vector.tensor_copy(b3rep[:], pb3[:])

        # timing-variant scratch: constant asb stand-in (pe_only)
        asb_const = None
        if variant == "pe_only":
            asb_const = consts.tile([P, CH], BF16)
            nc.gpsimd.memset(asb_const[:], 0.5)

        def prep_load(p):
            """x load + bf16 round — emitted early in the previous pair's
            pipeline so the DMA/Pool latency is hidden long before the
            PE-side prep_compute needs x_bf."""
            x_sb = xpool.tile([P, TBLK, F], F32, tag="x_sb", name=f"x_sb{p}")
            nc.sync.dma_start(
                x_sb[:], x_d[p].rearrange("(q t) f -> q t f", q=P)
            )
            # SBUF->SBUF, so Pool can own it (frees DVE/ACT for PSUM work)
            x_bf = xpool.tile([P, TBLK, F], BF16, tag="x_bf", name=f"x_bf{p}")
            nc.gpsimd.tensor_copy(x_bf[:], x_sb[:])
            return x_bf

        def prep_compute_steps(p, x_bf, out_st):
            """Parity transposes, e1/e2 (+ swapped-half copy of e1), split
            into 6 steps injected every other pipeline slot so the engine
            side-effects (xta/e-copies) never burst-clog DVE/ACT and the
            PSUM-pool punctures stay gentle.

            xT in parity layout: column (c', q) holds f@t=2c' on rows
            0..63 and f@t=2c'+1 on rows 64..127."""
            xta = xtpool.tile([P, NH], BF16, tag="xta", name=f"xta{p}")
            e1t = epool.tile([P, NH], BF16, tag="e1t", name=f"e1t{p}")
            e2t = epool.tile([P, NH], BF16, tag="e2t", name=f"e2t{p}")
            e1s = epool.tile([P, NH], BF16, tag="e1s", name=f"e1s{p}")

            def transposes(cps):
                def step():
                    for cp in cps:
                        pt = ps_adj.tile([P, P], BF16, tag="pa",
                                         name=f"pt{p}_{cp}")
                        nc.tensor.transpose(
                            pt[:], x_bf[:, 2 * cp : 2 * cp + 2, :], identb[:]
                        )
                        rot.copy(xta[:, ts(cp, P)], pt[:])
                return step

            def emms(wst, bc, et, last=False):
                # paired diagonal tiles (0,0)/(64,64); bias on the
                # PSUM->SBUF copy (per-partition bias AP, DVE low / ACT hi)
                def step():
                    for g in range(NCH2):
                        pe2 = ps_adj.tile([P, CH], F32, tag="pa",
                                          name=f"pe{p}_{g}")
                        nc.tensor.matmul(
                            pe2[0:HP, :], wst[0:HP, :], xta[0:HP, ts(g, CH)],
                            start=True, stop=True,
                        )
                        nc.tensor.matmul(
                            pe2[HP:P, :], wst[HP:P, :], xta[HP:P, ts(g, CH)],
                            start=True, stop=True,
                        )
                        nc.vector.tensor_scalar_add(
                            et[0:HP, ts(g, CH)], pe2[0:HP, :], bc[0:HP, :])
                        nc.scalar.activation(et[HP:P, ts(g, CH)],
                                             pe2[HP:P, :], ID,
                                             bias=bc[HP:P, :])
                    if last:
                        # swapped-half copy of e1 so any (row-parity,
                        # col-parity) adj instruction finds its moving
                        # operand on the right partitions.
                        nc.sync.dma_start(e1s[0:HP, :], e1t[HP:P, :])
                        nc.sync.dma_start(e1s[HP:P, :], e1t[0:HP, :])
                        out_st.append((x_bf, e1t, e1s, e2t))
                return step

            return [
                transposes((0, 1)), transposes((2, 3)),
                transposes((4, 5)), transposes((6, 7)),
                emms(w2st, b2c, e2t),
                emms(w1st, b1c, e1t, last=True),
            ]

        CHUNKS = [(pc, g) for g in range(NCH2) for pc in range(2)]
        SLOTS = [(u, pc, g) for u in range(NPAIR_T) for (pc, g) in CHUNKS]
        LAG = 4  # h-matmuls trail adjacency by one block-pair (4 slots)

        def main(p, st, tail_emit):
            """Flat software pipeline over 32 adjacency-chunk slots.
            Slot k emits: h for slot k-LAG, then the paired adjacency
            matmuls (row-tiles 0/64, concurrent on the PE) for slot k,
            then their relus (DVE / ACT). Every PE instruction's deps are
            >= LAG-1 slots old, so the PE queue never drains and pairing
            engages."""
            x_bf, e1t, e1s, e2t = st
            ph = [
                ps_h.tile([P, CH], F32, tag="ph", name=f"ph{p}_{g}")
                for g in range(NCH2)
            ]
            next_st = None
            next_xbf = None
            asbs = {}

            def emit_h(k):
                u, pc, g = SLOTS[k]
                for pt_ in range(2):
                    src = (asb_const if variant == "pe_only"
                           else asbs.pop((u, pc, g, pt_)))
                    # skip_group_check: the interp's PSUM group tracker
                    # is partition-blind; the two half-bank groups are
                    # on disjoint partitions (HW zeroes per element).
                    nc.tensor.matmul(
                        ph[g][HP * pc : HP * pc + HP, :],
                        x_bf[:, 2 * u + pt_, :],
                        src[:],
                        start=(u == 0 and pt_ == 0),
                        stop=(u == NPAIR_T - 1 and pt_ == 1),
                        skip_group_check=True,
                    )

            for k, (u, pc, g) in enumerate(SLOTS):
                if k >= LAG:
                    emit_h(k - LAG)
                pas = []
                for pt_ in range(2):  # t = 2u + pt_, row-tile 64*pt_
                    mv = e1t if pc == pt_ else e1s
                    pa = ps_adj.tile([P, CH], F32, tag="pa",
                                     name=f"pa{p}_{u}_{pc}_{g}_{pt_}")
                    nc.tensor.matmul(
                        pa[:],
                        e2t[HP * pt_ : HP * pt_ + HP, ts(u, P)],
                        mv[HP * pt_ : HP * pt_ + HP, ts(g, CH)],
                        start=True, stop=True,
                    )
                    pas.append(pa)
                for pt_ in range(2):
                    if variant == "pe_only":
                        continue
                    asb = adjpool.tile([P, CH], BF16, tag="asb",
                                       name=f"asb{p}_{u}_{pc}_{g}_{pt_}")
                    # deterministic per-slot engine split keeps DVE/ACT in
                    # lockstep so paired banks free together
                    if pt_ == 0:
                        nc.vector.tensor_scalar_max(asb[:], pas[pt_][:], 0.0)
                    else:
                        nc.scalar.activation(asb[:], pas[pt_][:], AF.Relu)
                    asbs[(u, pc, g, pt_)] = asb
                if k == 8 and tail_emit is not None:
                    tail_emit()
                    tail_emit = None
                if p + 1 < PAIRS:
                    if k == 2:
                        next_xbf = prep_load(p + 1)
                        next_acc = []
                        psteps = prep_compute_steps(p + 1, next_xbf, next_acc)
                    elif k in (12, 14, 16, 18, 20, 22):
                        psteps[(k - 12) // 2]()
                        if next_acc:
                            next_st = next_acc[0]
            for k in range(len(SLOTS) - LAG, len(SLOTS)):
                emit_h(k)

            # hT -> SBUF (parity layout [128, NH]): frees the ph banks.
            hta = htpool.tile([P, NH], BF16, tag="hta", name=f"hta{p}")
            for g in range(NCH2):
                rot.copy(hta[0:HP, ts(g, CH)], ph[g][0:HP, :])
                rot.copy(hta[HP:P, ts(g, CH)], ph[g][HP:P, :])

            def tail():
                # out = h @ W3^T + b3: per t-block, stationary
                # hta[parity-half, c'-block] (paired row tiles), moving
                # W3^T; + b3 fused into the PSUM->SBUF tensor-tensor add.
                # out_sb free layout [cp, par, o] == [(t) o] row-major
                out_sb = opool.tile([P, NPAIR_T, 2, O], F32, tag="out_sb",
                                    name=f"out_sb{p}")
                poe = ps_adj.tile([P, CH], F32, tag="pa", name=f"poe{p}")
                poo = ps_adj.tile([P, CH], F32, tag="pa", name=f"poo{p}")
                for cp in range(NPAIR_T):
                    nc.tensor.matmul(
                        poe[:, ts(cp, O)], hta[0:HP, ts(cp, P)],
                        w3st[0:HP, :], start=True, stop=True,
                    )
                    nc.tensor.matmul(
                        poo[:, ts(cp, O)], hta[HP:P, ts(cp, P)],
                        w3st[HP:P, :], start=True, stop=True,
                    )
                nc.vector.tensor_tensor(
                    out_sb[:, :, 0, :],
                    poe[:].rearrange("q (j o) -> q j o", o=O),
                    b3rep[:].rearrange("q (j o) -> q j o", o=O),
                    mybir.AluOpType.add,
                )
                nc.vector.tensor_tensor(
                    out_sb[:, :, 1, :],
                    poo[:].rearrange("q (j o) -> q j o", o=O),
                    b3rep[:].rearrange("q (j o) -> q j o", o=O),
                    mybir.AluOpType.add,
                )
                nc.sync.dma_start(
                    out_d[p].rearrange("(q cp par) f -> q cp par f",
                                       q=P, cp=NPAIR_T, par=2),
                    out_sb[:],
                )

            return next_st, tail

        def body():
            acc0 = []
            for step in prep_compute_steps(0, prep_load(0), acc0):
                step()
            st = acc0[0]
            tail = None
            for p in range(PAIRS):
                st, tail = main(p, st, tail)
            tail()

        if reps == 1:
            body()
        else:
            with tc.For_i(0, reps, 1):
                body()


def build_program(reps=1, variant=None):
    import os
    if variant is None:
        variant = os.environ.get("KVAR", "full")
    nc = bacc.Bacc("TRN2", target_bir_lowering=False, debug=False)
    x_d = nc.dram_tensor("x", [PAIRS, N, F], F32, kind="ExternalInput").ap()
    w_d = [
        nc.dram_tensor(f"w{k}", [H, F], F32, kind="ExternalInput").ap()
        for k in (1, 2, 3)
    ]
    b_d = [
        nc.dram_tensor(f"b{k}", [H], F32, kind="ExternalInput").ap()
        for k in (1, 2, 3)
    ]
    out_d = nc.dram_tensor("out", [PAIRS, N, O], F32, kind="ExternalOutput").ap()
    with tile.TileContext(nc) as tc:
        _emit(tc, x_d, w_d, b_d, out_d, reps=reps, variant=variant)
    nc.compile()
    return nc


def make_in_maps(x, W1, b1, W2, b2, W3, b3):
    xs = np.ascontiguousarray(np.asarray(x, np.float32).reshape(B * C, N, F))
    const = {
        "w1": np.ascontiguousarray(np.asarray(W1, np.float32)),
        "w2": np.ascontiguousarray(np.asarray(W2, np.float32)),
        "w3": np.ascontiguousarray(np.asarray(W3, np.float32)),
        "b1": np.ascontiguousarray(np.asarray(b1, np.float32)),
        "b2": np.ascontiguousarray(np.asarray(b2, np.float32)),
        "b3": np.ascontiguousarray(np.asarray(b3, np.float32)),
    }
    return [
        {"x": np.ascontiguousarray(xs[i * PAIRS : (i + 1) * PAIRS]), **const}
        for i in range(NCORES)
    ]


_NC_CACHE = {}


def kernel(x, W1, b1, W2, b2, W3, b3):
    from concourse.bass_utils import run_bass_kernel_spmd

    if "nc" not in _NC_CACHE:
        _NC_CACHE["nc"] = build_program()
    nc = _NC_CACHE["nc"]
    in_maps = make_in_maps(x, W1, b1, W2, b2, W3, b3)
    res = run_bass_kernel_spmd(nc, in_maps, list(range(NCORES))).results
    out = np.concatenate([res[i]["out"] for i in range(NCORES)], axis=0)
    return out.reshape(B, C, N, O)


# revision 27
# speedup vs baseline: 1.1121x; 1.1121x over previous
"""GCN message-passing kernel for 8 TRN2 NeuronCores.

Reference computation (per (b, c) pair, fp32 reference):
    e1  = x @ W1^T + b1          [N, H]
    e2  = x @ W2^T + b2          [N, H]
    adj = relu(e1 @ e2^T)        [N, N]
    h   = adj @ x                [N, F]
    out = h @ W3^T + b3          [N, O]

Sharding: the 32 (b, c) pairs are split 4-per-core across 8 cores;
weights are replicated. Each core runs an identical Bass program fully
fused in SBUF/PSUM (the N x N adjacency never touches HBM).

All matmul operands are bf16 (PSUM accumulation stays fp32); the 2e-2
rel-err gate leaves ample room (bf16 quantization contributes ~3e-3).

Parity layout: every K=64 contraction is packed two-per-PE via
tile_position row tiles (0,*) / (64,*), which the PE executes
concurrently (measured 2x). Row-block t of a pair lives on partition
half t%2: the [128,128] x-transposes emit (f@t-even ; f@t-odd) stacked,
and e1/e2/h^T inherit the layout, so adjacency, the e-matmuls and the
output projection all pair up. biases ride along on the mandatory
PSUM->SBUF copies (per-partition bias on ACT, tensor-tensor add on DVE)
instead of K=65 augmentation, keeping contractions pairable at K=64.

Layout trick: partition p of SBUF holds rows [16p, 16p+16) of the pair
(a pure row permutation, applied consistently to both sides of every
contraction and undone by the output DMA), which makes every HBM
transfer contiguous 4KB per partition.
"""

import sys

for _p in ("/opt/trn_rl_repo",):
    if _p not in sys.path:
        sys.path.insert(0, _p)

import numpy as np

import concourse.bass as bass
import concourse.tile as tile
from concourse import bacc, mybir
from concourse.bass import ts
from concourse.masks import make_identity

B, C, N, F = 4, 8, 2048, 64
H = 64
O = 64
NCORES = 8
PAIRS = (B * C) // NCORES  # 4 (b,c) pairs per core
P = 128                    # SBUF partitions
HP = 64                    # half-partition (PE row/col tile)
TBLK = N // P              # 16 row-blocks per pair
NPAIR_T = TBLK // 2        # 8 block-pairs (even t, odd t)
CH = 512                   # moving-operand chunk (one PSUM bank of fp32)
NH = N // 2                # 1024 columns per parity
NCH2 = NH // CH            # 2 chunks per parity
F32 = mybir.dt.float32
BF16 = mybir.dt.bfloat16

AF = mybir.ActivationFunctionType
ID = AF.Identity


class _Rotor:
    """Alternate PSUM->SBUF relu/copy work across DVE and ACT.
    (GPSIMD cannot access PSUM on TRN2, so Pool only gets SBUF-only work.)"""

    def __init__(self, nc):
        self.nc = nc
        self.i = 0

    def relu(self, out, in_):
        e = "da"[self.i % 2]
        self.i += 1
        if e == "d":
            self.nc.vector.tensor_scalar_max(out, in_, 0.0)
        else:
            self.nc.scalar.activation(out, in_, AF.Relu)

    def copy(self, out, in_):
        e = "da"[self.i % 2]
        self.i += 1
        if e == "d":
            self.nc.vector.tensor_copy(out, in_)
        else:
            self.nc.scalar.copy(out, in_)


def _emit(tc, x_d, w_d, b_d, out_d, reps=1, variant="full"):
    nc = tc.nc
    rot = _Rotor(nc)

    import contextlib

    with contextlib.ExitStack() as ctx:
        consts = ctx.enter_context(tc.tile_pool(name="consts", bufs=1))
        xpool = ctx.enter_context(tc.tile_pool(name="xp", bufs=2))
        xtpool = ctx.enter_context(tc.tile_pool(name="xt", bufs=2))
        epool = ctx.enter_context(tc.tile_pool(name="ep", bufs=2))
        adjpool = ctx.enter_context(tc.tile_pool(name="adj", bufs=12))
        htpool = ctx.enter_context(tc.tile_pool(name="ht", bufs=2))
        opool = ctx.enter_context(tc.tile_pool(name="op", bufs=2))
        ps_adj = ctx.enter_context(tc.tile_pool(name="psa", bufs=6, space="PSUM"))
        ps_h = ctx.enter_context(tc.tile_pool(name="psh", bufs=2, space="PSUM"))

        ident = consts.tile([P, P], F32)
        make_identity(nc, ident[:])
        identb = consts.tile([P, P], BF16)
        make_identity(nc, identb[:])

        # Weights W1/W2/W3 transposed to [f, h] bf16, duplicated on both
        # partition halves (for row-tile pairing); per-partition bias cols.
        wstats, bcols = [], []
        for k in range(3):
            wraw = consts.tile([H, F], F32, tag="wraw")
            nc.sync.dma_start(wraw[:], w_d[k][:])
            wst = consts.tile([P, H], BF16, tag=f"wst{k}")
            pw = ps_adj.tile([F, H], F32, tag="pa")
            nc.tensor.transpose(pw[:], wraw[:], ident[0:H, 0:H])
            nc.vector.tensor_copy(wst[0:HP, :], pw[:])
            nc.sync.dma_start(wst[HP:P, :], wst[0:HP, :])
            wstats.append(wst)
            if k < 2:
                bcol = consts.tile([P, 1], F32, tag=f"bc{k}")
                nc.sync.dma_start(bcol[0:HP, :], b_d[k].unsqueeze(1))
                nc.sync.dma_start(bcol[HP:P, :], b_d[k].unsqueeze(1))
                bcols.append(bcol)
        w1st, w2st, w3st = wstats
        b1c, b2c = bcols

        # b3 replicated [128, 512] (cols = 8 blocks x 64 o) for the fused
        # bias-add on the projection's PSUM->SBUF copy.
        b3row = consts.tile([1, O], F32)
        nc.sync.dma_start(b3row[:], b_d[2].unsqueeze(0))
        b3x8 = consts.tile([1, CH], F32)
        for j in range(CH // O):
            nc.vector.tensor_copy(b3x8[:, ts(j, O)], b3row[:])
        ones1 = consts.tile([1, P], F32)
        nc.gpsimd.memset(ones1[:], 1.0)
        b3rep = consts.tile([P, CH], F32)
        pb3 = ps_adj.tile([P, CH], F32, tag="pa")
        nc.tensor.matmul(pb3[:], ones1[:], b3x8[:], start=True, stop=True)
        nc.vector.tensor_copy(b3rep[:], pb3[:])

        # timing-variant scratch: constant asb stand-in (pe_only)
        asb_const = None
        if variant == "pe_only":
            asb_const = consts.tile([P, CH], BF16)
            nc.gpsimd.memset(asb_const[:], 0.5)

        def prep_load(p):
            """x load + bf16 round — emitted early in the previous pair's
            pipeline so the DMA/Pool latency is hidden long before the
            PE-side prep_compute needs x_bf."""
            x_sb = xpool.tile([P, TBLK, F], F32, tag="x_sb", name=f"x_sb{p}")
            nc.sync.dma_start(
                x_sb[:], x_d[p].rearrange("(q t) f -> q t f", q=P)
            )
            # SBUF->SBUF, so Pool can own it (frees DVE/ACT for PSUM work)
            x_bf = xpool.tile([P, TBLK, F], BF16, tag="x_bf", name=f"x_bf{p}")
            nc.gpsimd.tensor_copy(x_bf[:], x_sb[:])
            return x_bf

        def prep_compute_steps(p, x_bf, out_st):
            """Parity transposes, e1/e2 (+ swapped-half copy of e1), split
            into 6 steps injected every other pipeline slot so the engine
            side-effects (xta/e-copies) never burst-clog DVE/ACT and the
            PSUM-pool punctures stay gentle.

            xT in parity layout: column (c', q) holds f@t=2c' on rows
            0..63 and f@t=2c'+1 on rows 64..127."""
            xta = xtpool.tile([P, NH], BF16, tag="xta", name=f"xta{p}")
            e1t = epool.tile([P, NH], BF16, tag="e1t", name=f"e1t{p}")
            e2t = epool.tile([P, NH], BF16, tag="e2t", name=f"e2t{p}")
            e1s = epool.tile([P, NH], BF16, tag="e1s", name=f"e1s{p}")

            def transposes(cps):
                def step():
                    for cp in cps:
                        pt = ps_adj.tile([P, P], BF16, tag="pa",
                                         name=f"pt{p}_{cp}")
                        nc.tensor.transpose(
                            pt[:], x_bf[:, 2 * cp : 2 * cp + 2, :], identb[:]
                        )
                        rot.copy(xta[:, ts(cp, P)], pt[:])
                return step

            def emms(wst, bc, et, last=False):
                # paired diagonal tiles (0,0)/(64,64); bias on the
                # PSUM->SBUF copy (per-partition bias AP, DVE low / ACT hi)
                def step():
                    for g in range(NCH2):
                        pe2 = ps_adj.tile([P, CH], F32, tag="pa",
                                          name=f"pe{p}_{g}")
                        nc.tensor.matmul(
                            pe2[0:HP, :], wst[0:HP, :], xta[0:HP, ts(g, CH)],
                            start=True, stop=True,
                        )
                        nc.tensor.matmul(
                            pe2[HP:P, :], wst[HP:P, :], xta[HP:P, ts(g, CH)],
                            start=True, stop=True,
                        )
                        nc.vector.tensor_scalar_add(
                            et[0:HP, ts(g, CH)], pe2[0:HP, :], bc[0:HP, :])
                        nc.scalar.activation(et[HP:P, ts(g, CH)],
                                             pe2[HP:P, :], ID,
                                             bias=bc[HP:P, :])
                    if last:
                        # swapped-half copy of e1 so any (row-parity,
                        # col-parity) adj instruction finds its moving
                        # operand on the right partitions.
                        nc.sync.dma_start(e1s[0:HP, :], e1t[HP:P, :])
                        nc.sync.dma_start(e1s[HP:P, :], e1t[0:HP, :])
                        out_st.append((x_bf, e1t, e1s, e2t))
                return step

            return [
                transposes((0, 1)), transposes((2, 3)),
                transposes((4, 5)), transposes((6, 7)),
                emms(w2st, b2c, e2t),
                emms(w1st, b1c, e1t, last=True),
            ]

        import os
        CHUNKS = [(pc, g) for g in range(NCH2) for pc in range(2)]
        SLOTS = [(u, pc, g) for u in range(NPAIR_T) for (pc, g) in CHUNKS]
        # h-matmuls trail adjacency by LAG slots (>= one block-pair)
        LAG = int(os.environ.get("KLAG", "4"))

        def main(p, st, tail_emit):
            """Flat software pipeline over 32 adjacency-chunk slots.
            Slot k emits: h for slot k-LAG, then the paired adjacency
            matmuls (row-tiles 0/64, concurrent on the PE) for slot k,
            then their relus (DVE / ACT). Every PE instruction's deps are
            >= LAG-1 slots old, so the PE queue never drains and pairing
            engages."""
            x_bf, e1t, e1s, e2t = st
            ph = [
                ps_h.tile([P, CH], F32, tag="ph", name=f"ph{p}_{g}")
                for g in range(NCH2)
            ]
            next_st = None
            next_xbf = None
            next_acc = None
            psteps = None
            asbs = {}

            def emit_h(k):
                u, pc, g = SLOTS[k]
                for pt_ in range(2):
                    src = (asb_const if variant == "pe_only"
                           else asbs.pop((u, pc, g, pt_)))
                    # skip_group_check: the interp's PSUM group tracker
                    # is partition-blind; the two half-bank groups are
                    # on disjoint partitions (HW zeroes per element).
                    nc.tensor.matmul(
                        ph[g][HP * pc : HP * pc + HP, :],
                        x_bf[:, 2 * u + pt_, :],
                        src[:],
                        start=(u == 0 and pt_ == 0),
                        stop=(u == NPAIR_T - 1 and pt_ == 1),
                        skip_group_check=True,
                    )

            def emit_adj(k):
                u, pc, g = SLOTS[k]
                pas = []
                for pt_ in range(2):  # t = 2u + pt_, row-tile 64*pt_
                    mv = e1t if pc == pt_ else e1s
                    pa = ps_adj.tile([P, CH], F32, tag="pa",
                                     name=f"pa{p}_{u}_{pc}_{g}_{pt_}")
                    nc.tensor.matmul(
                        pa[:],
                        e2t[HP * pt_ : HP * pt_ + HP, ts(u, P)],
                        mv[HP * pt_ : HP * pt_ + HP, ts(g, CH)],
                        start=True, stop=True,
                    )
                    pas.append(pa)
                for pt_ in range(2):
                    if variant == "pe_only":
                        continue
                    asb = adjpool.tile([P, CH], BF16, tag="asb",
                                       name=f"asb{p}_{u}_{pc}_{g}_{pt_}")
                    # deterministic per-slot engine split keeps DVE/ACT in
                    # lockstep so paired banks free together
                    if pt_ == 0:
                        nc.vector.tensor_scalar_max(asb[:], pas[pt_][:], 0.0)
                    else:
                        nc.scalar.activation(asb[:], pas[pt_][:], AF.Relu)
                    asbs[(u, pc, g, pt_)] = asb

            def inject(k):
                nonlocal next_xbf, next_acc, psteps, next_st, tail_emit
                if k == 8 and tail_emit is not None:
                    tail_emit()
                    tail_emit = None
                if p + 1 < PAIRS:
                    if k == 2:
                        next_xbf = prep_load(p + 1)
                        next_acc = []
                        psteps = prep_compute_steps(p + 1, next_xbf, next_acc)
                    elif k in (12, 14, 16, 18, 20, 22):
                        psteps[(k - 12) // 2]()
                        if next_acc:
                            next_st = next_acc[0]

            if variant == "useg":
                # u-granularity: runs of 8 h (K=128, full-array mode) then
                # runs of 8 adj (K=64 row-tile pairs) — minimizes PE
                # tile-mode switches at the cost of shorter dep slack.
                for u in range(NPAIR_T):
                    base = 4 * u
                    if u >= 1:
                        for k in range(base - 4, base):
                            emit_h(k)
                    for k in range(base, base + 4):
                        emit_adj(k)
                    inject(base)
                    inject(base + 2)
                for k in range(len(SLOTS) - 4, len(SLOTS)):
                    emit_h(k)
            else:
                for k in range(len(SLOTS)):
                    if k >= LAG:
                        emit_h(k - LAG)
                    emit_adj(k)
                    inject(k)
                for k in range(len(SLOTS) - LAG, len(SLOTS)):
                    emit_h(k)

            # hT -> SBUF (parity layout [128, NH]): frees the ph banks.
            hta = htpool.tile([P, NH], BF16, tag="hta", name=f"hta{p}")
            for g in range(NCH2):
                rot.copy(hta[0:HP, ts(g, CH)], ph[g][0:HP, :])
                rot.copy(hta[HP:P, ts(g, CH)], ph[g][HP:P, :])

            def tail():
                # out = h @ W3^T + b3: per t-block, stationary
                # hta[parity-half, c'-block] (paired row tiles), moving
                # W3^T; + b3 fused into the PSUM->SBUF tensor-tensor add.
                # out_sb free layout [cp, par, o] == [(t) o] row-major
                out_sb = opool.tile([P, NPAIR_T, 2, O], F32, tag="out_sb",
                                    name=f"out_sb{p}")
                poe = ps_adj.tile([P, CH], F32, tag="pa", name=f"poe{p}")
                poo = ps_adj.tile([P, CH], F32, tag="pa", name=f"poo{p}")
                for cp in range(NPAIR_T):
                    nc.tensor.matmul(
                        poe[:, ts(cp, O)], hta[0:HP, ts(cp, P)],
                        w3st[0:HP, :], start=True, stop=True,
                    )
                    nc.tensor.matmul(
                        poo[:, ts(cp, O)], hta[HP:P, ts(cp, P)],
                        w3st[HP:P, :], start=True, stop=True,
                    )
                nc.vector.tensor_tensor(
                    out_sb[:, :, 0, :],
                    poe[:].rearrange("q (j o) -> q j o", o=O),
                    b3rep[:].rearrange("q (j o) -> q j o", o=O),
                    mybir.AluOpType.add,
                )
                nc.vector.tensor_tensor(
                    out_sb[:, :, 1, :],
                    poo[:].rearrange("q (j o) -> q j o", o=O),
                    b3rep[:].rearrange("q (j o) -> q j o", o=O),
                    mybir.AluOpType.add,
                )
                nc.sync.dma_start(
                    out_d[p].rearrange("(q cp par) f -> q cp par f",
                                       q=P, cp=NPAIR_T, par=2),
                    out_sb[:],
                )

            return next_st, tail

        def body():
            acc0 = []
            for step in prep_compute_steps(0, prep_load(0), acc0):
                step()
            st = acc0[0]
            tail = None
            for p in range(PAIRS):
                st, tail = main(p, st, tail)
            tail()

        if reps == 1:
            body()
        else:
            with tc.For_i(0, reps, 1):
                body()


def build_program(reps=1, variant=None):
    import os
    if variant is None:
        variant = os.environ.get("KVAR", "full")
    nc = bacc.Bacc("TRN2", target_bir_lowering=False, debug=False)
    x_d = nc.dram_tensor("x", [PAIRS, N, F], F32, kind="ExternalInput").ap()
    w_d = [
        nc.dram_tensor(f"w{k}", [H, F], F32, kind="ExternalInput").ap()
        for k in (1, 2, 3)
    ]
    b_d = [
        nc.dram_tensor(f"b{k}", [H], F32, kind="ExternalInput").ap()
        for k in (1, 2, 3)
    ]
    out_d = nc.dram_tensor("out", [PAIRS, N, O], F32, kind="ExternalOutput").ap()
    with tile.TileContext(nc) as tc:
        _emit(tc, x_d, w_d, b_d, out_d, reps=reps, variant=variant)
    nc.compile()
    return nc


def make_in_maps(x, W1, b1, W2, b2, W3, b3):
    xs = np.ascontiguousarray(np.asarray(x, np.float32).reshape(B * C, N, F))
    const = {
        "w1": np.ascontiguousarray(np.asarray(W1, np.float32)),
        "w2": np.ascontiguousarray(np.asarray(W2, np.float32)),
        "w3": np.ascontiguousarray(np.asarray(W3, np.float32)),
        "b1": np.ascontiguousarray(np.asarray(b1, np.float32)),
        "b2": np.ascontiguousarray(np.asarray(b2, np.float32)),
        "b3": np.ascontiguousarray(np.asarray(b3, np.float32)),
    }
    return [
        {"x": np.ascontiguousarray(xs[i * PAIRS : (i + 1) * PAIRS]), **const}
        for i in range(NCORES)
    ]


_NC_CACHE = {}


def kernel(x, W1, b1, W2, b2, W3, b3):
    from concourse.bass_utils import run_bass_kernel_spmd

    if "nc" not in _NC_CACHE:
        _NC_CACHE["nc"] = build_program()
    nc = _NC_CACHE["nc"]
    in_maps = make_in_maps(x, W1, b1, W2, b2, W3, b3)
    res = run_bass_kernel_spmd(nc, in_maps, list(range(NCORES))).results
    out = np.concatenate([res[i]["out"] for i in range(NCORES)], axis=0)
    return out.reshape(B, C, N, O)


# revision 29
# speedup vs baseline: 1.6143x; 1.4516x over previous
"""GCN message-passing kernel for 8 TRN2 NeuronCores.

Reference computation (per (b, c) pair, fp32 reference):
    e1  = x @ W1^T + b1          [N, H]
    e2  = x @ W2^T + b2          [N, H]
    adj = relu(e1 @ e2^T)        [N, N]
    h   = adj @ x                [N, F]
    out = h @ W3^T + b3          [N, O]

Sharding: the 32 (b, c) pairs are split 4-per-core across 8 cores;
weights are replicated. Each core runs an identical Bass program fully
fused in SBUF/PSUM (the N x N adjacency never touches HBM).

All matmul operands are bf16 (PSUM accumulation stays fp32); the 2e-2
rel-err gate leaves ample room (bf16 quantization contributes ~3e-3).

Parity layout: every K=64 contraction is packed two-per-PE via
tile_position row tiles (0,*) / (64,*), which the PE executes
concurrently (measured 2x). Row-block t of a pair lives on partition
half t%2: the [128,128] x-transposes emit (f@t-even ; f@t-odd) stacked,
and e1/e2/h^T inherit the layout, so adjacency, the e-matmuls and the
output projection all pair up. biases ride along on the mandatory
PSUM->SBUF copies (per-partition bias on ACT, tensor-tensor add on DVE)
instead of K=65 augmentation, keeping contractions pairable at K=64.

Layout trick: partition p of SBUF holds rows [16p, 16p+16) of the pair
(a pure row permutation, applied consistently to both sides of every
contraction and undone by the output DMA), which makes every HBM
transfer contiguous 4KB per partition.
"""

import sys

for _p in ("/opt/trn_rl_repo",):
    if _p not in sys.path:
        sys.path.insert(0, _p)

import numpy as np

import concourse.bass as bass
import concourse.tile as tile
from concourse import bacc, mybir
from concourse.bass import ts
from concourse.masks import make_identity

B, C, N, F = 4, 8, 2048, 64
H = 64
O = 64
NCORES = 8
PAIRS = (B * C) // NCORES  # 4 (b,c) pairs per core
P = 128                    # SBUF partitions
HP = 64                    # half-partition (PE row/col tile)
TBLK = N // P              # 16 row-blocks per pair
NPAIR_T = TBLK // 2        # 8 block-pairs (even t, odd t)
CH = 512                   # moving-operand chunk (one PSUM bank of fp32)
NH = N // 2                # 1024 columns per parity
NCH2 = NH // CH            # 2 chunks per parity
F32 = mybir.dt.float32
BF16 = mybir.dt.bfloat16

AF = mybir.ActivationFunctionType
ID = AF.Identity


class _Rotor:
    """Alternate PSUM->SBUF relu/copy work across DVE and ACT.
    (GPSIMD cannot access PSUM on TRN2, so Pool only gets SBUF-only work.)"""

    def __init__(self, nc):
        self.nc = nc
        self.i = 0

    def relu(self, out, in_):
        e = "da"[self.i % 2]
        self.i += 1
        if e == "d":
            self.nc.vector.tensor_scalar_max(out, in_, 0.0)
        else:
            self.nc.scalar.activation(out, in_, AF.Relu)

    def copy(self, out, in_):
        e = "da"[self.i % 2]
        self.i += 1
        if e == "d":
            self.nc.vector.tensor_copy(out, in_)
        else:
            self.nc.scalar.copy(out, in_)


def _emit(tc, x_d, w_d, b_d, out_d, reps=1, variant="full"):
    nc = tc.nc
    rot = _Rotor(nc)

    import contextlib

    with contextlib.ExitStack() as ctx:
        consts = ctx.enter_context(tc.tile_pool(name="consts", bufs=1))
        xpool = ctx.enter_context(tc.tile_pool(name="xp", bufs=2))
        xtpool = ctx.enter_context(tc.tile_pool(name="xt", bufs=2))
        epool = ctx.enter_context(tc.tile_pool(name="ep", bufs=2))
        adjpool = ctx.enter_context(tc.tile_pool(name="adj", bufs=12))
        htpool = ctx.enter_context(tc.tile_pool(name="ht", bufs=2))
        opool = ctx.enter_context(tc.tile_pool(name="op", bufs=2))
        ps_adj = ctx.enter_context(tc.tile_pool(name="psa", bufs=6, space="PSUM"))
        ps_h = ctx.enter_context(tc.tile_pool(name="psh", bufs=2, space="PSUM"))

        ident = consts.tile([P, P], F32)
        make_identity(nc, ident[:])
        identb = consts.tile([P, P], BF16)
        make_identity(nc, identb[:])

        # Weights W1/W2/W3 transposed to [f, h] bf16, duplicated on both
        # partition halves (for row-tile pairing); per-partition bias cols.
        wstats, bcols = [], []
        for k in range(3):
            wraw = consts.tile([H, F], F32, tag="wraw")
            nc.sync.dma_start(wraw[:], w_d[k][:])
            wst = consts.tile([P, H], BF16, tag=f"wst{k}")
            pw = ps_adj.tile([F, H], F32, tag="pa")
            nc.tensor.transpose(pw[:], wraw[:], ident[0:H, 0:H])
            nc.vector.tensor_copy(wst[0:HP, :], pw[:])
            nc.sync.dma_start(wst[HP:P, :], wst[0:HP, :])
            wstats.append(wst)
            if k < 2:
                bcol = consts.tile([P, 1], F32, tag=f"bc{k}")
                nc.sync.dma_start(bcol[0:HP, :], b_d[k].unsqueeze(1))
                nc.sync.dma_start(bcol[HP:P, :], b_d[k].unsqueeze(1))
                bcols.append(bcol)
        w1st, w2st, w3st = wstats
        b1c, b2c = bcols

        # b3 replicated [128, 512] (cols = 8 blocks x 64 o) for the fused
        # bias-add on the projection's PSUM->SBUF copy.
        b3row = consts.tile([1, O], F32)
        nc.sync.dma_start(b3row[:], b_d[2].unsqueeze(0))
        b3x8 = consts.tile([1, CH], F32)
        for j in range(CH // O):
            nc.vector.tensor_copy(b3x8[:, ts(j, O)], b3row[:])
        ones1 = consts.tile([1, P], F32)
        nc.gpsimd.memset(ones1[:], 1.0)
        b3rep = consts.tile([P, CH], F32)
        pb3 = ps_adj.tile([P, CH], F32, tag="pa")
        nc.tensor.matmul(pb3[:], ones1[:], b3x8[:], start=True, stop=True)
        nc.vector.tensor_copy(b3rep[:], pb3[:])

        # timing-variant scratch: constant asb stand-in (pe_only)
        asb_const = None
        if variant == "pe_only":
            asb_const = consts.tile([P, CH], BF16)
            nc.gpsimd.memset(asb_const[:], 0.5)

        def prep_load(p):
            """x load + bf16 round — emitted early in the previous pair's
            pipeline so the DMA/Pool latency is hidden long before the
            PE-side prep_compute needs x_bf."""
            x_sb = xpool.tile([P, TBLK, F], F32, tag="x_sb", name=f"x_sb{p}")
            nc.sync.dma_start(
                x_sb[:], x_d[p].rearrange("(q t) f -> q t f", q=P)
            )
            # SBUF->SBUF, so Pool can own it (frees DVE/ACT for PSUM work)
            x_bf = xpool.tile([P, TBLK, F], BF16, tag="x_bf", name=f"x_bf{p}")
            nc.gpsimd.tensor_copy(x_bf[:], x_sb[:])
            return x_bf

        def prep_compute_steps(p, x_bf, out_st):
            """Parity transposes, e1/e2 (+ swapped-half copy of e1), split
            into 6 steps injected every other pipeline slot so the engine
            side-effects (xta/e-copies) never burst-clog DVE/ACT and the
            PSUM-pool punctures stay gentle.

            xT in parity layout: column (c', q) holds f@t=2c' on rows
            0..63 and f@t=2c'+1 on rows 64..127."""
            xta = xtpool.tile([P, NH], BF16, tag="xta", name=f"xta{p}")
            e1t = epool.tile([P, NH], BF16, tag="e1t", name=f"e1t{p}")
            e2t = epool.tile([P, NH], BF16, tag="e2t", name=f"e2t{p}")
            e1s = epool.tile([P, NH], BF16, tag="e1s", name=f"e1s{p}")

            def transposes(cps):
                def step():
                    for cp in cps:
                        pt = ps_adj.tile([P, P], BF16, tag="pa",
                                         name=f"pt{p}_{cp}")
                        nc.tensor.transpose(
                            pt[:], x_bf[:, 2 * cp : 2 * cp + 2, :], identb[:]
                        )
                        rot.copy(xta[:, ts(cp, P)], pt[:])
                return step

            def emms(wst, bc, et, last=False):
                # paired diagonal tiles (0,0)/(64,64); bias on the
                # PSUM->SBUF copy (per-partition bias AP, DVE low / ACT hi)
                def step():
                    for g in range(NCH2):
                        pe2 = ps_adj.tile([P, CH], F32, tag="pa",
                                          name=f"pe{p}_{g}")
                        nc.tensor.matmul(
                            pe2[0:HP, :], wst[0:HP, :], xta[0:HP, ts(g, CH)],
                            start=True, stop=True,
                        )
                        nc.tensor.matmul(
                            pe2[HP:P, :], wst[HP:P, :], xta[HP:P, ts(g, CH)],
                            start=True, stop=True,
                        )
                        nc.vector.tensor_scalar_add(
                            et[0:HP, ts(g, CH)], pe2[0:HP, :], bc[0:HP, :])
                        nc.scalar.activation(et[HP:P, ts(g, CH)],
                                             pe2[HP:P, :], ID,
                                             bias=bc[HP:P, :])
                    if last:
                        # swapped-half copy of e1 so any (row-parity,
                        # col-parity) adj instruction finds its moving
                        # operand on the right partitions.
                        nc.sync.dma_start(e1s[0:HP, :], e1t[HP:P, :])
                        nc.sync.dma_start(e1s[HP:P, :], e1t[0:HP, :])
                        out_st.append((x_bf, e1t, e1s, e2t))
                return step

            return [
                transposes((0, 1)), transposes((2, 3)),
                transposes((4, 5)), transposes((6, 7)),
                emms(w2st, b2c, e2t),
                emms(w1st, b1c, e1t, last=True),
            ]

        import os
        CHUNKS = [(pc, g) for g in range(NCH2) for pc in range(2)]
        SLOTS = [(u, pc, g) for u in range(NPAIR_T) for (pc, g) in CHUNKS]
        # h-matmuls trail adjacency by LAG slots (>= one block-pair)
        LAG = int(os.environ.get("KLAG", "4"))

        def main(p, st, tail_emit):
            """Flat software pipeline over 32 adjacency-chunk slots.
            Slot k emits: h for slot k-LAG, then the paired adjacency
            matmuls (row-tiles 0/64, concurrent on the PE) for slot k,
            then their relus (DVE / ACT). Every PE instruction's deps are
            >= LAG-1 slots old, so the PE queue never drains and pairing
            engages."""
            x_bf, e1t, e1s, e2t = st
            ph = [
                ps_h.tile([P, CH], F32, tag="ph", name=f"ph{p}_{g}")
                for g in range(NCH2)
            ]
            next_st = None
            next_xbf = None
            next_acc = None
            psteps = None
            asbs = {}

            def emit_h(k):
                u, pc, g = SLOTS[k]
                for pt_ in range(2):
                    src = (asb_const if variant == "pe_only"
                           else asbs.pop((u, pc, g, pt_)))
                    # skip_group_check: the interp's PSUM group tracker
                    # is partition-blind; the two half-bank groups are
                    # on disjoint partitions (HW zeroes per element).
                    nc.tensor.matmul(
                        ph[g][HP * pc : HP * pc + HP, :],
                        x_bf[:, 2 * u + pt_, :],
                        src[:],
                        start=(u == 0 and pt_ == 0),
                        stop=(u == NPAIR_T - 1 and pt_ == 1),
                        skip_group_check=True,
                    )

            def emit_adj(k):
                u, pc, g = SLOTS[k]
                pas = []
                for pt_ in range(2):  # t = 2u + pt_, row-tile 64*pt_
                    mv = e1t if pc == pt_ else e1s
                    pa = ps_adj.tile([P, CH], F32, tag="pa",
                                     name=f"pa{p}_{u}_{pc}_{g}_{pt_}")
                    nc.tensor.matmul(
                        pa[:],
                        e2t[HP * pt_ : HP * pt_ + HP, ts(u, P)],
                        mv[HP * pt_ : HP * pt_ + HP, ts(g, CH)],
                        start=True, stop=True,
                    )
                    pas.append(pa)
                for pt_ in range(2):
                    if variant == "pe_only":
                        continue
                    asb = adjpool.tile([P, CH], BF16, tag="asb",
                                       name=f"asb{p}_{u}_{pc}_{g}_{pt_}")
                    # deterministic per-slot engine split keeps DVE/ACT in
                    # lockstep so paired banks free together
                    if pt_ == 0:
                        nc.vector.tensor_scalar_max(asb[:], pas[pt_][:], 0.0)
                    else:
                        nc.scalar.activation(asb[:], pas[pt_][:], AF.Relu)
                    asbs[(u, pc, g, pt_)] = asb

            def inject(k):
                nonlocal next_xbf, next_acc, psteps, next_st, tail_emit
                if k == 8 and tail_emit is not None:
                    tail_emit()
                    tail_emit = None
                if p + 1 < PAIRS:
                    if k == 2:
                        next_xbf = prep_load(p + 1)
                        next_acc = []
                        psteps = prep_compute_steps(p + 1, next_xbf, next_acc)
                    elif k in (12, 14, 16, 18, 20, 22):
                        psteps[(k - 12) // 2]()
                        if next_acc:
                            next_st = next_acc[0]

            if variant in ("useg", "pe_only"):
                # u-granularity: runs of 8 h (K=128, full-array mode) then
                # runs of 8 adj (K=64 row-tile pairs) — minimizes PE
                # tile-mode switches at the cost of shorter dep slack.
                # Injections sit between the runs: transposes (full-array)
                # after the h-run, proj pairs (row-tiles) before the adj run.
                for u in range(NPAIR_T):
                    base = 4 * u
                    if u >= 1:
                        for k in range(base - 4, base):
                            emit_h(k)
                    inject(base)
                    inject(base + 2)
                    for k in range(base, base + 4):
                        emit_adj(k)
                for k in range(len(SLOTS) - 4, len(SLOTS)):
                    emit_h(k)
            else:
                for k in range(len(SLOTS)):
                    if k >= LAG:
                        emit_h(k - LAG)
                    emit_adj(k)
                    inject(k)
                for k in range(len(SLOTS) - LAG, len(SLOTS)):
                    emit_h(k)

            # hT -> SBUF (parity layout [128, NH]): frees the ph banks.
            hta = htpool.tile([P, NH], BF16, tag="hta", name=f"hta{p}")
            for g in range(NCH2):
                rot.copy(hta[0:HP, ts(g, CH)], ph[g][0:HP, :])
                rot.copy(hta[HP:P, ts(g, CH)], ph[g][HP:P, :])

            def tail():
                # out = h @ W3^T + b3: per t-block, stationary
                # hta[parity-half, c'-block] (paired row tiles), moving
                # W3^T; + b3 fused into the PSUM->SBUF tensor-tensor add.
                # out_sb free layout [cp, par, o] == [(t) o] row-major
                out_sb = opool.tile([P, NPAIR_T, 2, O], F32, tag="out_sb",
                                    name=f"out_sb{p}")
                poe = ps_adj.tile([P, CH], F32, tag="pa", name=f"poe{p}")
                poo = ps_adj.tile([P, CH], F32, tag="pa", name=f"poo{p}")
                for cp in range(NPAIR_T):
                    nc.tensor.matmul(
                        poe[:, ts(cp, O)], hta[0:HP, ts(cp, P)],
                        w3st[0:HP, :], start=True, stop=True,
                    )
                    nc.tensor.matmul(
                        poo[:, ts(cp, O)], hta[HP:P, ts(cp, P)],
                        w3st[HP:P, :], start=True, stop=True,
                    )
                nc.vector.tensor_tensor(
                    out_sb[:, :, 0, :],
                    poe[:].rearrange("q (j o) -> q j o", o=O),
                    b3rep[:].rearrange("q (j o) -> q j o", o=O),
                    mybir.AluOpType.add,
                )
                nc.vector.tensor_tensor(
                    out_sb[:, :, 1, :],
                    poo[:].rearrange("q (j o) -> q j o", o=O),
                    b3rep[:].rearrange("q (j o) -> q j o", o=O),
                    mybir.AluOpType.add,
                )
                nc.sync.dma_start(
                    out_d[p].rearrange("(q cp par) f -> q cp par f",
                                       q=P, cp=NPAIR_T, par=2),
                    out_sb[:],
                )

            return next_st, tail

        def body():
            acc0 = []
            for step in prep_compute_steps(0, prep_load(0), acc0):
                step()
            st = acc0[0]
            tail = None
            for p in range(PAIRS):
                st, tail = main(p, st, tail)
            tail()

        if reps == 1:
            body()
        else:
            with tc.For_i(0, reps, 1):
                body()


def build_program(reps=1, variant=None):
    import os
    if variant is None:
        variant = os.environ.get("KVAR", "useg")
    nc = bacc.Bacc("TRN2", target_bir_lowering=False, debug=False)
    x_d = nc.dram_tensor("x", [PAIRS, N, F], F32, kind="ExternalInput").ap()
    w_d = [
        nc.dram_tensor(f"w{k}", [H, F], F32, kind="ExternalInput").ap()
        for k in (1, 2, 3)
    ]
    b_d = [
        nc.dram_tensor(f"b{k}", [H], F32, kind="ExternalInput").ap()
        for k in (1, 2, 3)
    ]
    out_d = nc.dram_tensor("out", [PAIRS, N, O], F32, kind="ExternalOutput").ap()
    with tile.TileContext(nc) as tc:
        _emit(tc, x_d, w_d, b_d, out_d, reps=reps, variant=variant)
    nc.compile()
    return nc


def make_in_maps(x, W1, b1, W2, b2, W3, b3):
    xs = np.ascontiguousarray(np.asarray(x, np.float32).reshape(B * C, N, F))
    const = {
        "w1": np.ascontiguousarray(np.asarray(W1, np.float32)),
        "w2": np.ascontiguousarray(np.asarray(W2, np.float32)),
        "w3": np.ascontiguousarray(np.asarray(W3, np.float32)),
        "b1": np.ascontiguousarray(np.asarray(b1, np.float32)),
        "b2": np.ascontiguousarray(np.asarray(b2, np.float32)),
        "b3": np.ascontiguousarray(np.asarray(b3, np.float32)),
    }
    return [
        {"x": np.ascontiguousarray(xs[i * PAIRS : (i + 1) * PAIRS]), **const}
        for i in range(NCORES)
    ]


_NC_CACHE = {}


def kernel(x, W1, b1, W2, b2, W3, b3):
    from concourse.bass_utils import run_bass_kernel_spmd

    if "nc" not in _NC_CACHE:
        _NC_CACHE["nc"] = build_program()
    nc = _NC_CACHE["nc"]
    in_maps = make_in_maps(x, W1, b1, W2, b2, W3, b3)
    res = run_bass_kernel_spmd(nc, in_maps, list(range(NCORES))).results
    out = np.concatenate([res[i]["out"] for i in range(NCORES)], axis=0)
    return out.reshape(B, C, N, O)
